# revision 5
# baseline (speedup 1.0000x reference)
"""CompGCN layer on 8 Trainium2 NeuronCores (Bass/Tile).

Strategy (edge-parallel, output-row sharded):
  - Forward messages are sharded by target entity range (core c owns rows
    [6250c, 6250(c+1))); backward messages by source range. Each core owns its
    6250-row output slice, so no big cross-core reduction is needed.
  - Per-edge gather of x_e rows uses dma_gather (int16 indices; the 50000-row
    table is addressed as two halves, <32768 and >=32768).
  - The scatter-add is computed race-free on the tensor engine: per 128-edge
    chunk, a weighted one-hot matrix P'[j, t] = w_j * (tgt_j == t) is built
    on-chip with one fused DVE tensor_scalar (is_equal, mult), and
    acc[f, t] += msgs_chunk^T @ P' accumulates in PSUM per entity tile.
  - The x_r term never gathers: it is a dense matmul x_r^T @ S_T where
    S_T[r, t] = -sum of edge weights with (relation r, local target t),
    precomputed on host purely from edge_index/edge_type (index metadata).
  - Epilogue fuses the three weight matmuls in PSUM (feature-major), computes
    BatchNorm statistics with ACT accum_out, all-reduces the [128,2] partial
    stats across the 8 cores, applies the affine, transposes via PE, and
    writes the output slice. x_r_new = x_r @ w_rel.T is computed per-core.

The program structure is identical across cores (SPMD); all per-core variation
is in the input data (indices, weights, S_T, x_e slice). Per-(direction, tile,
half) chunk counts are the max over the 8 cores, so every core runs the same
instruction stream with zero-weight padding slots.
"""

import numpy as np

N_ENT = 50000
N_REL = 500
N_EDGE = 600000
DIM = 128
CORES = 8
SLICE = N_ENT // CORES          # 6250
TILES = (SLICE + 127) // 128    # 49
SLICE_PAD = TILES * 128         # 6272
LAST_ROWS = SLICE - (TILES - 1) * 128  # 106
HALF_SPLIT = 32768
NRELPAD = 1024                  # 2*N_REL = 1000 padded to 8*128
BLK = 32                        # chunks per gather call (4096 slots)
BN_EPS = 1e-5

_CACHE = {}


def _wrap_idx_blocks(idx_slots, nch_list):
    """Pack per-stream gather indices into the [128, S/16] int16 layout the
    Q7 cores expect: per gather call (block), slot j sits at partition j%16,
    column (block col base + j//16), replicated across the 8 Q7 core groups."""
    total_cols = sum(nch_list) * 8
    out = np.zeros((16, total_cols), np.int16)
    col = 0
    slot = 0
    for nch in nch_list:
        n = nch * 128
        blkcols = n // 16
        blk = idx_slots[slot:slot + n].reshape(blkcols, 16).T
        out[:, col:col + blkcols] = blk
        col += blkcols
        slot += n
    return np.tile(out, (8, 1)).copy()


def _preprocess(edge_index, edge_type):
    src = np.asarray(edge_index[0], np.int64)
    tgt = np.asarray(edge_index[1], np.int64)
    et = np.asarray(edge_type, np.int64)
    deg_s = np.bincount(src, minlength=N_ENT).astype(np.float64)
    deg_t = np.bincount(tgt, minlength=N_ENT).astype(np.float64)
    w = (1.0 / np.sqrt(deg_s[src] * deg_t[tgt])).astype(np.float32)

    # per (dir, core): edge subsets
    # dir 0 (fwd): shard by tgt, gather src, relation 2*et
    # dir 1 (bwd): shard by src, gather tgt, relation 2*et+1
    dirs = []
    for d, (tkey, gkey, rel) in enumerate(
        [(tgt, src, 2 * et), (src, tgt, 2 * et + 1)]
    ):
        core_of = tkey // SLICE
        tloc = tkey - core_of * SLICE
        tile = tloc // 128
        tmod = tloc % 128
        half = (gkey >= HALF_SPLIT).astype(np.int64)
        dirs.append((core_of, tloc, tile, tmod, half, gkey, rel))

    # uniform chunk counts per (dir, tile, half) = ceil(max over cores / 128)
    nch = np.zeros((2, TILES, 2), np.int64)
    counts = np.zeros((2, CORES, TILES, 2), np.int64)
    for d in range(2):
        core_of, _, tile, _, half, _, _ = dirs[d]
        key = ((core_of * TILES + tile) * 2 + half)
        cnt = np.bincount(key, minlength=CORES * TILES * 2).reshape(CORES, TILES, 2)
        counts[d] = cnt
        nch[d] = (cnt.max(axis=0) + 127) // 128

    # stream layout per (dir, half): tile-major chunk ranges
    meta = {"nch": nch, "tile_ch": [], "NCH": np.zeros((2, 2), np.int64)}
    for d in range(2):
        tc = []
        bases = np.zeros((TILES, 2), np.int64)
        for h in range(2):
            b = 0
            for t in range(TILES):
                bases[t, h] = b
                b += nch[d, t, h]
            meta["NCH"][d, h] = b
        meta["tile_ch"].append(bases)

    # per-core packed arrays
    per_core = []
    for c in range(CORES):
        cin = {}
        for d in range(2):
            core_of, tloc, tile, tmod, half, gkey, rel = dirs[d]
            sel = np.nonzero(core_of == c)[0]
            NCH_lo, NCH_hi = meta["NCH"][d]
            S_lo, S_hi = NCH_lo * 128, NCH_hi * 128
            gidx = [np.zeros(S_lo, np.int64), np.zeros(S_hi, np.int64)]
            tmod_arr = np.zeros((2, max(S_lo, S_hi)), np.float32)
            w_arr = np.zeros((2, max(S_lo, S_hi)), np.float32)
            for h in range(2):
                hsel = sel[half[sel] == h]
                order = np.argsort(tile[hsel], kind="stable")
                hsel = hsel[order]
                # slot positions: tile-run bases + within-run offset
                t_of = tile[hsel]
                run_off = np.zeros(len(hsel), np.int64)
                if len(hsel):
                    # within-tile sequence number
                    run_off = np.arange(len(hsel)) - np.searchsorted(
                        t_of, t_of, side="left"
                    )
                slots = meta["tile_ch"][d][t_of, h] * 128 + run_off
                gv = gkey[hsel] - (HALF_SPLIT if h else 0)
                gidx[h][slots] = gv
                tmod_arr[h][slots] = tmod[hsel].astype(np.float32)
                w_arr[h][slots] = w[hsel]
            # idx wrapping per gather block
            blocks = [
                [min(BLK, int(meta["NCH"][d][h]) - b) for b in
                 range(0, int(meta["NCH"][d][h]), BLK)]
                for h in range(2)
            ]
            cin[f"gidx{d}lo"] = _wrap_idx_blocks(gidx[0], blocks[0])
            cin[f"gidx{d}hi"] = _wrap_idx_blocks(gidx[1], blocks[1])
            # tmod/w tiles: [128, NCH_lo + NCH_hi]; col = chunk, row = slot%128
            NCHT = int(NCH_lo + NCH_hi)
            tm = np.zeros((128, NCHT), np.float32)
            wv = np.zeros((128, NCHT), np.float32)
            tm[:, :NCH_lo] = tmod_arr[0][:S_lo].reshape(NCH_lo, 128).T
            wv[:, :NCH_lo] = w_arr[0][:S_lo].reshape(NCH_lo, 128).T
            tm[:, NCH_lo:] = tmod_arr[1][:S_hi].reshape(NCH_hi, 128).T
            wv[:, NCH_lo:] = w_arr[1][:S_hi].reshape(NCH_hi, 128).T
            cin[f"tmod{d}"] = tm
            cin[f"wval{d}"] = wv
            # S_T: [NRELPAD, SLICE_PAD], negated aggregated weights
            st = np.zeros((NRELPAD, SLICE_PAD), np.float32)
            np.add.at(st, (rel[sel], tloc[sel]), -w[sel].astype(np.float64))
            cin[f"st{d}"] = st
        per_core.append(cin)
    return meta, per_core


def _build_program(meta):
    import concourse.bacc as bacc
    import concourse.tile as tile_mod
    import concourse.mybir as mybir

    f32 = mybir.dt.float32
    i16 = mybir.dt.int16
    AF = mybir.ActivationFunctionType
    ALU = mybir.AluOpType

    nch = meta["nch"]
    NCH = meta["NCH"]
    tile_ch = meta["tile_ch"]

    nc = bacc.Bacc("TRN2", target_bir_lowering=False, debug=False,
                   num_devices=CORES, num_swdge_queues=4)

    def inp(name, shape, dt=f32):
        return nc.dram_tensor(name, shape, dt, kind="ExternalInput").ap()

    x_e = inp("x_e", [N_ENT, DIM])
    xe_slice = inp("xe_slice", [SLICE_PAD, DIM])
    x_r = inp("x_r", [NRELPAD, DIM])
    w_loop = inp("w_loop", [DIM, DIM])
    w_fwd = inp("w_fwd", [DIM, DIM])
    w_bwd = inp("w_bwd", [DIM, DIM])
    w_relT = inp("w_relT", [DIM, DIM])
    self_col = inp("self_col", [DIM, 1])
    bias_col = inp("bias_col", [DIM, 1])
    gamma_col = inp("gamma_col", [DIM, 1])
    beta_col = inp("beta_col", [DIM, 1])
    iota_in = inp("iota", [DIM, DIM])
    ident_in = inp("ident", [DIM, DIM])
    gidx_in = {}
    for d in range(2):
        for h, hn in enumerate(("lo", "hi")):
            gidx_in[(d, h)] = inp(f"gidx{d}{hn}", [128, int(NCH[d][h]) * 8], i16)
    tmod_in = [inp(f"tmod{d}", [128, int(NCH[d][0] + NCH[d][1])]) for d in range(2)]
    wval_in = [inp(f"wval{d}", [128, int(NCH[d][0] + NCH[d][1])]) for d in range(2)]
    st_in = [inp(f"st{d}", [NRELPAD, SLICE_PAD]) for d in range(2)]

    out_slice = nc.dram_tensor("out_slice", [SLICE_PAD, DIM], f32,
                               kind="ExternalOutput").ap()
    xr_new = nc.dram_tensor("xr_new", [NRELPAD, DIM], f32,
                            kind="ExternalOutput").ap()

    with tile_mod.TileContext(nc) as tc:
        with (
            tc.tile_pool(name="const", bufs=1) as cpool,
            tc.tile_pool(name="accp", bufs=1) as accpool,
            tc.tile_pool(name="meta", bufs=1) as mpool,
            tc.tile_pool(name="blk0", bufs=2) as blkpool_lo,
            tc.tile_pool(name="blk1", bufs=2) as blkpool_hi,
            tc.tile_pool(name="ph", bufs=6) as ppool,
            tc.tile_pool(name="stp", bufs=2) as stpool,
            tc.tile_pool(name="ep", bufs=4) as epool,
            tc.tile_pool(name="ps", bufs=4, space="PSUM") as pspool,
            tc.tile_pool(name="ps2", bufs=2, space="PSUM") as pspool2,
            tc.tile_pool(name="dram", bufs=2, space="DRAM") as dpool,
        ):
            # ---- constants into SBUF
            def load(pool, ap, shape, dt=f32, name=None):
                name = name or ap.tensor.name
                t = pool.tile(shape, dt, name=f"sb_{name}", tag=f"sb_{name}")
                nc.sync.dma_start(t[:], ap[:])
                return t

            iota = load(cpool, iota_in, [128, 128])
            ident = load(cpool, ident_in, [128, 128])
            wl_sb = load(cpool, w_loop, [128, 128])
            wf_sb = load(cpool, w_fwd, [128, 128])
            wb_sb = load(cpool, w_bwd, [128, 128])
            wrT_sb = load(cpool, w_relT, [128, 128])
            self_sb = load(cpool, self_col, [128, 1])
            bias_sb = load(cpool, bias_col, [128, 1])
            gamma_sb = load(cpool, gamma_col, [128, 1])
            beta_sb = load(cpool, beta_col, [128, 1])
            xr_sb = cpool.tile([128, 8, 128], f32)
            nc.sync.dma_start(
                xr_sb[:], x_r.rearrange("(s p) f -> p s f", p=128)[:]
            )
            tmod_sb = [load(mpool, tmod_in[d],
                            [128, int(NCH[d][0] + NCH[d][1])]) for d in range(2)]
            wval_sb = [load(mpool, wval_in[d],
                            [128, int(NCH[d][0] + NCH[d][1])]) for d in range(2)]
            gidx_sb = {}
            for d in range(2):
                for h in range(2):
                    gidx_sb[(d, h)] = load(
                        mpool, gidx_in[(d, h)], [128, int(NCH[d][h]) * 8], i16
                    )

            acc_sb = [accpool.tile([128, SLICE_PAD], f32, tag=f"acc{i}", name=f"acc{i}") for i in range(2)]

            # ---- main message-passing accumulation
            qrot = [0]

            def gather_block(d, h, b, nch_b):
                pool = blkpool_lo if h == 0 else blkpool_hi
                t = pool.tile([128, BLK, 128], f32, tag=f"blk{h}")
                base = x_e[0:HALF_SPLIT, :] if h == 0 else x_e[HALF_SPLIT:N_ENT, :]
                n = nch_b * 128
                idx = gidx_sb[(d, h)][:, b * BLK * 8: b * BLK * 8 + n // 16]
                nc.gpsimd.dma_gather(
                    t[:, 0:nch_b, :], base, idx, n, n, 128,
                    single_packet=False, queue_num=qrot[0] % 4,
                )
                qrot[0] += 1
                return t

            cur_blk = {}   # (d,h) -> (b, tile)
            for d in range(2):
                NCH_lo = int(NCH[d][0])
                # total matmuls per (d, t): 8 + nch[d,t,0] + nch[d,t,1]
                for t in range(TILES):
                    ps = pspool.tile([128, 128], f32, tag="msum")
                    stt = stpool.tile([128, 8, 128], f32)
                    nc.sync.dma_start(
                        stt[:],
                        st_in[d].rearrange("(s p) t -> p s t", p=128)
                        [:, :, 128 * t: 128 * t + 128],
                    )
                    n_lo = int(nch[d, t, 0])
                    n_hi = int(nch[d, t, 1])
                    n_ops = 8 + n_lo + n_hi
                    op_i = 0
                    for s in range(8):
                        nc.tensor.matmul(
                            ps[:], xr_sb[:, s:s+1, :], stt[:, s:s+1, :],
                            start=(op_i == 0), stop=(op_i == n_ops - 1),
                        )
                        op_i += 1
                    for h, n_h in ((0, n_lo), (1, n_hi)):
                        cbase = int(tile_ch[d][t, h])
                        for k in range(n_h):
                            cg = cbase + k
                            b, cib = divmod(cg, BLK)
                            key = (d, h)
                            if cur_blk.get(key, (-1, None))[0] != b:
                                nb = min(BLK, int(NCH[d][h]) - b * BLK)
                                cur_blk[key] = (b, gather_block(d, h, b, nb))
                            blkt = cur_blk[key][1]
                            col = cg if h == 0 else NCH_lo + cg
                            P = ppool.tile([128, 128], f32, tag="ph")
                            nc.vector.tensor_scalar(
                                P[:], iota[:],
                                tmod_sb[d][:, col:col + 1],
                                wval_sb[d][:, col:col + 1],
                                ALU.is_equal, ALU.mult,
                            )
                            nc.tensor.matmul(
                                ps[:], blkt[:, cib:cib+1, :], P[:],
                                start=False, stop=(op_i == n_ops - 1),
                            )
                            op_i += 1
                    nc.vector.tensor_copy(acc_sb[d][:, 128 * t:128 * t + 128], ps[:])

            # ---- epilogue
            # cT[f'] = sum_f w_loop[f, f'] * self_loop[f]
            ps_c = pspool2.tile([128, 1], f32, tag="eps1")
            nc.tensor.matmul(ps_c[:], wl_sb[:], self_sb[:], start=True, stop=True)
            cT = epool.tile([128, 1], f32, tag="cT")
            nc.vector.tensor_copy(cT[:], ps_c[:])
            b1 = epool.tile([128, 1], f32, tag="b1")
            # b1 = bias - cT/3
            nc.vector.tensor_scalar(b1[:], cT[:], -1.0 / 3.0, None, ALU.mult)
            nc.vector.tensor_add(b1[:], b1[:], bias_sb[:])

            s1cols = epool.tile([128, TILES], f32, tag="s1")
            s2cols = epool.tile([128, TILES], f32, tag="s2")
            junk = epool.tile([128, 128], f32, tag="junk")

            for t in range(TILES):
                # xeT tile: load [128e, 128f] slice, transpose to [128f, 128e]
                xet = epool.tile([128, 128], f32, tag="xet")
                nc.sync.dma_start(xet[:], xe_slice[128 * t:128 * t + 128, :])
                ps_tr = pspool2.tile([128, 128], f32, tag="eps2")
                nc.tensor.transpose(ps_tr[:], xet[:], ident[:])
                xetT = epool.tile([128, 128], f32, tag="xetT")
                nc.vector.tensor_copy(xetT[:], ps_tr[:])

                ps_e = pspool2.tile([128, 128], f32, tag="eps2")
                nc.tensor.matmul(ps_e[:], wl_sb[:], xetT[:], start=True, stop=False)
                nc.tensor.matmul(ps_e[:], wf_sb[:],
                                 acc_sb[0][:, 128 * t:128 * t + 128],
                                 start=False, stop=False)
                nc.tensor.matmul(ps_e[:], wb_sb[:],
                                 acc_sb[1][:, 128 * t:128 * t + 128],
                                 start=False, stop=True)
                ncols = 128 if t < TILES - 1 else LAST_ROWS
                # pre = psum/3 + b1 ; write back into acc_sb[0] slot (reuse)
                pre = acc_sb[0][:, 128 * t:128 * t + ncols]
                nc.scalar.activation(pre, ps_e[:, 0:ncols], AF.Identity,
                                     bias=b1[:], scale=1.0 / 3.0,
                                     accum_out=s1cols[:, t:t + 1])
                nc.scalar.activation(junk[:, 0:ncols], pre, AF.Square,
                                     accum_out=s2cols[:, t:t + 1])

            # stats: S1, S2 -> AllReduce across cores
            stats = epool.tile([128, 2], f32, tag="stats")
            nc.vector.tensor_reduce(stats[:, 0:1], s1cols[:],
                                    mybir.AxisListType.X, ALU.add)
            nc.vector.tensor_reduce(stats[:, 1:2], s2cols[:],
                                    mybir.AxisListType.X, ALU.add)
            cc_in = dpool.tile([128, 2], f32)
            cc_out = dpool.tile([128, 2], f32)
            nc.gpsimd.dma_start(cc_in[:], stats[:])
            nc.gpsimd.collective_compute(
                "AllReduce", ALU.add,
                replica_groups=[list(range(CORES))],
                ins=[cc_in.opt()], outs=[cc_out.opt()],
            )
            gstats = epool.tile([128, 2], f32, tag="gstats")
            nc.gpsimd.dma_start(gstats[:], cc_out[:])

            mean = epool.tile([128, 1], f32, tag="mean")
            nc.vector.tensor_scalar(mean[:], gstats[:, 0:1], 1.0 / N_ENT, None,
                                    ALU.mult)
            ex2 = epool.tile([128, 1], f32, tag="ex2")
            nc.vector.tensor_scalar(ex2[:], gstats[:, 1:2], 1.0 / N_ENT, None,
                                    ALU.mult)
            var = epool.tile([128, 1], f32, tag="var")
            nc.vector.tensor_mul(var[:], mean[:], mean[:])
            nc.vector.tensor_sub(var[:], ex2[:], var[:])
            nc.vector.tensor_scalar(var[:], var[:], float(BN_EPS), None, ALU.add)
            sd = epool.tile([128, 1], f32, tag="sd")
            nc.scalar.activation(sd[:], var[:], AF.Sqrt)
            rsd = epool.tile([128, 1], f32, tag="rsd")
            nc.vector.reciprocal(rsd[:], sd[:])
            a_col = epool.tile([128, 1], f32, tag="acol")
            nc.vector.tensor_mul(a_col[:], gamma_sb[:], rsd[:])
            b2 = epool.tile([128, 1], f32, tag="b2")
            nc.vector.tensor_mul(b2[:], mean[:], a_col[:])
            nc.vector.tensor_sub(b2[:], beta_sb[:], b2[:])

            for t in range(TILES):
                fin = epool.tile([128, 128], f32, tag="fin")
                nc.scalar.activation(fin[:], acc_sb[0][:, 128 * t:128 * t + 128],
                                     AF.Identity, bias=b2[:], scale=a_col[:])
                ps_o = pspool2.tile([128, 128], f32, tag="eps2")
                nc.tensor.transpose(ps_o[:], fin[:], ident[:])
                ent = epool.tile([128, 128], f32, tag="ent")
                nc.vector.tensor_copy(ent[:], ps_o[:])
                rows = 128 if t < TILES - 1 else LAST_ROWS
                nc.sync.dma_start(out_slice[128 * t:128 * t + rows, :],
                                  ent[0:rows, :])

            # x_r_new = x_r @ w_rel.T
            for rt in range(8):
                ps_t = pspool2.tile([128, 128], f32, tag="eps2")
                nc.tensor.transpose(ps_t[:], xr_sb[:, rt:rt+1, :], ident[:])
                xrT = epool.tile([128, 128], f32, tag="xrT")
                nc.vector.tensor_copy(xrT[:], ps_t[:])
                ps_n = pspool2.tile([128, 128], f32, tag="eps2")
                nc.tensor.matmul(ps_n[:], xrT[:], wrT_sb[:], start=True, stop=True)
                xrn = epool.tile([128, 128], f32, tag="xrn")
                nc.vector.tensor_copy(xrn[:], ps_n[:])
                nc.sync.dma_start(xr_new[128 * rt:128 * rt + 128, :], xrn[:])

    nc.compile()
    return nc


def kernel(x_e, x_r, w_loop, w_fwd, w_bwd, w_rel, self_loop, bias,
           bn_gamma, bn_beta, edge_index, edge_type):
    from concourse import bass_utils

    x_e = np.asarray(x_e, np.float32)
    x_r = np.asarray(x_r, np.float32)
    meta, per_core = _preprocess(np.asarray(edge_index), np.asarray(edge_type))

    nc = _build_program(meta)

    x_r_pad = np.zeros((NRELPAD, DIM), np.float32)
    x_r_pad[: 2 * N_REL] = x_r
    common = {
        "x_e": x_e,
        "x_r": x_r_pad,
        "w_loop": np.asarray(w_loop, np.float32),
        "w_fwd": np.asarray(w_fwd, np.float32),
        "w_bwd": np.asarray(w_bwd, np.float32),
        "w_relT": np.ascontiguousarray(np.asarray(w_rel, np.float32).T),
        "self_col": np.ascontiguousarray(
            np.asarray(self_loop, np.float32).reshape(DIM, 1)),
        "bias_col": np.asarray(bias, np.float32).reshape(DIM, 1).copy(),
        "gamma_col": np.asarray(bn_gamma, np.float32).reshape(DIM, 1).copy(),
        "beta_col": np.asarray(bn_beta, np.float32).reshape(DIM, 1).copy(),
        "iota": np.tile(np.arange(DIM, dtype=np.float32)[None, :], (DIM, 1)),
        "ident": np.eye(DIM, dtype=np.float32),
    }
    in_maps = []
    for c in range(CORES):
        m = dict(common)
        m["xe_slice"] = np.zeros((SLICE_PAD, DIM), np.float32)
        m["xe_slice"][:SLICE] = x_e[c * SLICE:(c + 1) * SLICE]
        m.update(per_core[c])
        in_maps.append(m)

    res = bass_utils.run_bass_kernel_spmd(nc, in_maps, core_ids=list(range(CORES)))
    out = np.concatenate(
        [res.results[c]["out_slice"][:SLICE] for c in range(CORES)], axis=0
    )
    x_r_new = res.results[0]["xr_new"][: 2 * N_REL].copy()
    return out, x_r_new
